# revision 13
# baseline (speedup 1.0000x reference)
"""Trainium2 Bass kernel for nn_Attention_73289321939579.

Gated attention block (AlphaFold-style):
  qkv = q_x @ w_qkv.T ; q /= sqrt(64)
  scores = q k^T + bias ; attn = softmax(scores, keys)
  o = (attn @ v) * sigmoid(q_x @ w_g.T + b_g)
  out = o @ w_o.T + b_o

Sharding over 8 cores: core = b*4 + qh*2 + hq
  b  = batch (2)            -> data parallel
  qh = query half (2x1024)  -> bias/q sliced, output row-sliced
  hq = head quad (2x4 heads)-> tensor parallel; partial outputs summed on host

v3 structure: ONE tiling-mode switch (mode switches drain the PE array):
  A (128x128): warmup + all projections (v projection emits both pairs
    at N=256). All host inputs are PRE-TILED so every dma_start reads a
    contiguous block (row-strided sources cost per-row descriptors and
    stretched the v2 DMA lead-in to ~25us).
  B (64x128 row-tiled): attention AND epilogue. S^T per head: K=64 head
    dims -> tiles T0/T8 run concurrently (227ns/pair measured). O: K=128
    keys ping-pongs lo/hi halves into separate banks per head
    (same-bank alternating-tile accumulation is FATAL on hw) and is
    summed at block end via DVE copy+add (TensorTensor reads at most
    one PSUM operand). hp-blocked: 2x2 O banks + 2 S slabs = 8 banks.
    The epilogue runs row-tiled too (reciprocal broadcast via a
    selection row on tile T8 over prezeroed fp32r tiles; output
    projection halves summed like O), so epilogue(0) is emitted between
    attention blocks and hides under the exp-bound window.
Elementwise: exp on ACT (the kernel bottleneck: 64 x [128,1024] fp32
psum -> bf16 sbuf, ~68us; the ACT queue carries NOTHING else - in v2
the bias-fetch DIRECT2D descriptors on this queue cost 40us). Bias
multiply on DVE in bf16. exp(bias) is precomputed on host, shipped
bf16 pre-tiled, and kept RESIDENT per query-chunk (2MB) - 8 grouped
sync-queue DMAs total instead of 64 scalar-queue fetches.
"""

import sys

for _p in ("/opt/trn_rl_repo",):
    if _p not in sys.path:
        sys.path.insert(0, _p)

import numpy as np
import ml_dtypes

import concourse.bass as bass  # noqa: F401
import concourse.mybir as mybir
import concourse.tile as tile
from concourse import bacc
from concourse.bass_utils import run_bass_kernel_spmd

# ---- problem dims (hardcoded per contest contract) ----
B, Q, CQ = 2, 2048, 512
H, D = 8, 64
P = 128
QL = 1024          # queries per core
EL = 256           # e-dims per core (4 heads x 64)
HL = 4             # heads per core
CC = CQ // P       # 4 contraction chunks over channels
EC = EL // P       # 2 head-pairs
NJ = Q // P        # 16 key chunks
NI = QL // 512     # 2 query chunks of 512
NG = 4             # eb dma groups per query chunk (4 key-chunks each)

F32 = mybir.dt.float32
F32R = mybir.dt.float32r
BF16 = mybir.dt.bfloat16
MUL = mybir.AluOpType.mult
ADD = mybir.AluOpType.add
EXP = mybir.ActivationFunctionType.Exp
TANH = mybir.ActivationFunctionType.Tanh

# wt_in quarter order (DMA/consumption order): K, Q, V, G
OFF_K, OFF_Q, OFF_V, OFF_G = 0, EL, 2 * EL, 3 * EL
QUARTER_OFF = [OFF_K, OFF_Q, OFF_V, OFF_G]


def _r(ap):
    """float32r view for matmul operands (single-pass fp22 on the PE)."""
    return ap.bitcast(F32R)


def _emit(tc, xt, ebt, wt, wot, bg, sel_in, outp):
    nc = tc.nc
    from contextlib import ExitStack

    with ExitStack() as ctx:
        const = ctx.enter_context(tc.tile_pool(name="const", bufs=1))
        biasp = ctx.enter_context(tc.tile_pool(name="biasp", bufs=2))
        # xT/wT (phase A only) share this pool with the deep es/pt rings
        # (phase B only) so the allocator reuses the same SBUF bytes
        bigp = ctx.enter_context(tc.tile_pool(name="bigp", bufs=1))
        workp = ctx.enter_context(tc.tile_pool(name="workp", bufs=2))
        odp = ctx.enter_context(tc.tile_pool(name="odp", bufs=4))
        psum = ctx.enter_context(tc.tile_pool(name="psum", bufs=2, space="PSUM"))

        # ---- small constants FIRST so compute can start immediately ----
        sel_sb = const.tile([P, P], F32R, name="sel_sb", tag="sel_sb")
        nc.sync.dma_start(sel_sb, sel_in)
        bg_sb = const.tile([P, EC], F32, name="bg_sb", tag="bg_sb")
        nc.sync.dma_start(bg_sb, bg)
        woT_sb = const.tile([P, EC, CQ], F32R, name="woT_sb", tag="woT_sb")
        nc.sync.dma_start(woT_sb, wot)

        # ---- bulk inputs; every source block is contiguous in DRAM ----
        wT_sb = bigp.tile([P, CC, 4 * EL], F32R, name="wT_sb", tag="wT_sb")
        for q4 in range(4):
            off = QUARTER_OFF[q4]
            for c in range(CC):
                nc.sync.dma_start(wT_sb[:, c, off : off + EL], wt[q4, c])
        xT_sb = bigp.tile([P, CC, Q], F32R, name="xT_sb", tag="xT_sb")
        for j4 in range(Q // 512):
            for c in range(CC):
                nc.sync.dma_start(
                    xT_sb[:, c, j4 * 512 : (j4 + 1) * 512], xt[j4, c]
                )

        # ---- resident intermediates ----
        kT_sb = const.tile([P, EC, Q], F32R, name="kT_sb", tag="kT_sb")
        q_sb = const.tile([P, EC, QL], F32R, name="q_sb", tag="q_sb")
        gp_sb = const.tile([P, EC, QL], F32, name="gp_sb", tag="gp_sb")
        og_sb = const.tile([P, EC, QL], F32R, name="og_sb", tag="og_sb")
        # V augmented with a ones column: [keychunk-part, jc, head, 65], bf16
        va_sb = const.tile([P, NJ, HL, D + 1], BF16, name="va_sb", tag="va_sb")
        nc.vector.memset(va_sb[:, :, :, D], 1.0)
        # prezeroed fp32r reciprocal tiles: the ACT rounding copy writes rows
        # 0:65 only; rows 65:128 stay zero so the row-tiled broadcast matmul
        # (T8, K=64) sees zeros outside the selection row
        recr_sbs = []
        for ri in range(2):
            rr = const.tile([P, 512], F32R, name=f"recr{ri}", tag=f"recr{ri}")
            nc.vector.memset(rr.bitcast(F32), 0.0)
            recr_sbs.append(rr)

        # ================= phase A: 128x128 =================
        # warmup burst: covers the pre-tiled DMA lead-in (~5us) + HAM ramp
        warm_ps = psum.tile([P, 2, 512], F32, tag="s", name="warm_ps")
        for wi in range(44):
            nc.tensor.matmul(
                warm_ps[:, 0, 0:P], _r(sel_sb), _r(sel_sb),
                start=(wi == 0), stop=(wi == 43),
            )
        warm_sb = workp.tile([P, P], F32, name="warm_sb", tag="recf")
        nc.vector.tensor_copy(out=warm_sb[:, 0:P], in_=warm_ps[:, 0, 0:P])

        def emit_k_pair(ec):
            for j4 in range(Q // 512):
                sl = slice(j4 * 512, (j4 + 1) * 512)
                ps_k = psum.tile([P, 2, 512], F32, tag="s", name="ps_k")
                for c in range(CC):
                    nc.tensor.matmul(
                        ps_k[:, 0, :],
                        _r(wT_sb[:, c, OFF_K + ec * P : OFF_K + (ec + 1) * P]),
                        _r(xT_sb[:, c, sl]),
                        start=(c == 0), stop=(c == CC - 1),
                    )
                nc.vector.tensor_copy(out=kT_sb[:, ec, sl], in_=ps_k[:, 0, :])

        def emit_q_pair(ec):
            for icc in range(NI):
                sl = slice(icc * 512, (icc + 1) * 512)
                ps_q = psum.tile([P, 2, 512], F32, tag="s", name="ps_q")
                for c in range(CC):
                    nc.tensor.matmul(
                        ps_q[:, 0, :],
                        _r(wT_sb[:, c, OFF_Q + ec * P : OFF_Q + (ec + 1) * P]),
                        _r(xT_sb[:, c, sl]),
                        start=(c == 0), stop=(c == CC - 1),
                    )
                nc.vector.tensor_copy(out=q_sb[:, ec, sl], in_=ps_q[:, 0, :])

        def emit_v_all():
            # both pairs at once: out [keys 128, 256 e-dims] per key chunk
            for jc in range(NJ):
                ps_v = psum.tile([P, 2, 512], F32, tag="s", name="ps_v")
                for c in range(CC):
                    nc.tensor.matmul(
                        ps_v[:, 0, 0:EL],
                        _r(xT_sb[:, c, jc * P : (jc + 1) * P]),
                        _r(wT_sb[:, c, OFF_V : OFF_V + EL]),
                        start=(c == 0), stop=(c == CC - 1),
                    )
                nc.vector.tensor_copy(
                    out=va_sb[:, jc, :, 0:D],
                    in_=ps_v[:, 0, 0:EL].rearrange("p (h d) -> p h d", h=HL),
                )

        def emit_gate_pair(ec):
            # sigmoid(x wg^T + bg) = 0.5*tanh(0.5 x wg^T + 0.5 bg) + 0.5
            # (Tanh shares the ACT table set with Exp -> no table swap)
            for icc in range(NI):
                sl = slice(icc * 512, (icc + 1) * 512)
                ps_g = psum.tile([P, 2, 512], F32, tag="s", name="ps_g")
                for c in range(CC):
                    nc.tensor.matmul(
                        ps_g[:, 0, :],
                        _r(wT_sb[:, c, OFF_G + ec * P : OFF_G + (ec + 1) * P]),
                        _r(xT_sb[:, c, sl]),
                        start=(c == 0), stop=(c == CC - 1),
                    )
                nc.scalar.activation(
                    gp_sb[:, ec, sl], ps_g[:, 0, :], TANH,
                    bias=bg_sb[:, ec : ec + 1], scale=0.5,
                )

        emit_k_pair(0)
        emit_q_pair(0)
        emit_v_all()
        emit_k_pair(1)
        emit_q_pair(1)
        emit_gate_pair(0)
        emit_gate_pair(1)
        nc.vector.tensor_scalar(gp_sb, gp_sb, 0.5, 0.5, MUL, ADD)

        # ================= phase B: 64x128 row-tiled =================
        o_sb_all = {}  # (ic, h) -> [65, 512] f32 numerator + denominator row

        def emit_attention(ic, hp, tail=False):
            """16 key chunks for query chunk ic, head pair hp (heads 2hp,2hp+1)."""
            isl = slice(ic * 512, (ic + 1) * 512)
            o_ps = [
                psum.tile([P, 512], F32, tag="o", name=f"o_ps{ic}_{hp}_{i}", bufs=4)
                for i in range(4)
            ]  # [h_even lo, h_even hi, h_odd lo, h_odd hi]
            for jc in range(NJ):
                # bias tile rides the idle sync HWDGE queue (on the ACT queue
                # the descriptor gen serialized with exp and cost 40us)
                eb_sb = biasp.tile([P, 512], BF16, name="eb_sb", tag="eb", bufs=3)
                nc.sync.dma_start(eb_sb, ebt[ic, jc])
                s_ps = psum.tile([P, 2, 512], F32, tag="s", name="s_ps")
                nc.tensor.matmul(
                    s_ps[:, 0, :],
                    _r(kT_sb[0:64, hp, jc * P : (jc + 1) * P]),
                    _r(q_sb[0:64, hp, isl]),
                    start=True, stop=True,
                )
                nc.tensor.matmul(
                    s_ps[:, 1, :],
                    _r(kT_sb[64:128, hp, jc * P : (jc + 1) * P]),
                    _r(q_sb[64:128, hp, isl]),
                    start=True, stop=True,
                )
                es_sb = bigp.tile([P, 2, 512], BF16, name="es_sb", tag="es", bufs=8)
                nc.scalar.activation(
                    es_sb.rearrange("p a b -> p (a b)"),
                    s_ps.rearrange("p a b -> p (a b)"), EXP,
                )
                pt_sb = bigp.tile([P, 2, 512], BF16, name="pt_sb", tag="pt", bufs=8)
                ebb = eb_sb[:, None, :].to_broadcast([P, 2, 512])
                nc.vector.tensor_tensor(pt_sb, es_sb, ebb, MUL)
                for hh in range(2):
                    h = 2 * hp + hh
                    nc.tensor.matmul(
                        o_ps[2 * hh + 0][0:65, :],
                        va_sb[0:64, jc, h, :],
                        pt_sb[0:64, hh, :],
                        start=(jc == 0), stop=(jc == NJ - 1),
                    )
                    nc.tensor.matmul(
                        o_ps[2 * hh + 1][0:65, :],
                        va_sb[64:128, jc, h, :],
                        pt_sb[64:128, hh, :],
                        start=(jc == 0), stop=(jc == NJ - 1),
                    )
            for hh in range(2):
                h = 2 * hp + hh
                # lo + hi halves: TensorTensor reads at most one psum operand,
                # so stage the hi bank through SBUF (DVE; ACT stays exp-only)
                ohi = workp.tile([65, 512], F32, name="ohi", tag="ohi")
                if tail:
                    nc.scalar.copy(ohi, o_ps[2 * hh + 1][0:65, :])
                else:
                    nc.vector.tensor_copy(out=ohi, in_=o_ps[2 * hh + 1][0:65, :])
                osb = odp.tile([65, 512], F32, name=f"o_sb{ic}_{h}", tag="od",
                               bufs=8)
                nc.vector.tensor_tensor(
                    osb, o_ps[2 * hh + 0][0:65, :], ohi, ADD
                )
                o_sb_all[(ic, h)] = osb

        outr = outp  # [NI, P, 4, CQ] pre-tiled output

        def emit_epilogue(ic, tail=False):
            """normalize + gate + output projection (row-tiled, phase B)."""
            isl = slice(ic * 512, (ic + 1) * 512)
            og_stg = workp.tile([64, EC, 512], F32, name="og_stg", tag="ogstg")
            ocp_sb = workp.tile([P, EC, 512], F32, name="ocp_sb", tag="ocp")
            for h in range(HL):
                hp, hh = h // 2, h % 2
                o_sb = o_sb_all[(ic, h)]
                recf_sb = workp.tile([P, 512], F32, name="recf_sb", tag="recf")
                # approx recip over the whole [65,512] accumulator (single-row
                # slices miscompute in the custom-DVE path); row 64 holds the
                # softmax denominators
                nc.vector.reciprocal_approx_fast(out=recf_sb[0:65, :], in_=o_sb)
                # fp32r rounding for the matmul operand (BIR requirement)
                recr_sb = recr_sbs[h % 2]
                nc.vector.tensor_copy(out=recr_sb[0:65, :], in_=recf_sb[0:65, :])
                # broadcast row 64 to all partitions: T8 matmul, stationary =
                # sel rows 64:128 (indicator at relative row 0, zeros below),
                # moving = recr rows 64:128 (rec at relative row 0, prezeroed
                # below)
                bc_ps = psum.tile([P, 2, 512], F32, tag="s", name="bc_ps")
                nc.tensor.matmul(
                    bc_ps[:, 0, :], _r(sel_sb[64:128, :]), recr_sb[64:128, :],
                    start=True, stop=True,
                )
                oc_dst = ocp_sb[0:64, hp, :] if hh == 0 else og_stg[:, hp, :]
                nc.vector.tensor_tensor(
                    oc_dst, bc_ps[0:64, 0, :], o_sb[0:64, :], MUL
                )
            nc.sync.dma_start(ocp_sb[64:128, :, :], og_stg)
            for hp in range(EC):
                nc.vector.tensor_tensor(
                    og_sb[:, hp, isl], ocp_sb[:, hp, :], gp_sb[:, hp, isl], MUL
                )
            # output projection: K=256 split into 64-row quarters; T0 chains
            # the lo halves, T8 the hi halves, DVE sums the two banks
            out_sb = odp.tile([P, 4, CQ], F32, name="out_sb", tag="outsb", bufs=2)
            for ip4 in range(4):
                ip = ic * 4 + ip4
                slab = psum.tile([P, 2, 512], F32, tag="s", name="ps_o")
                for ec in range(EC):
                    nc.tensor.matmul(
                        slab[:, 0, :],
                        _r(og_sb[0:64, ec, ip * P : (ip + 1) * P]),
                        _r(woT_sb[0:64, ec, :]),
                        start=(ec == 0), stop=(ec == EC - 1),
                    )
                    nc.tensor.matmul(
                        slab[:, 1, :],
                        _r(og_sb[64:128, ec, ip * P : (ip + 1) * P]),
                        _r(woT_sb[64:128, ec, :]),
                        start=(ec == 0), stop=(ec == EC - 1),
                    )
                ohi2 = workp.tile([P, 512], F32, name="ohi2", tag="ohi2")
                if tail:
                    nc.scalar.copy(ohi2, slab[:, 1, :])
                else:
                    nc.vector.tensor_copy(out=ohi2, in_=slab[:, 1, :])
                nc.vector.tensor_tensor(
                    out_sb[:, ip4, :], slab[:, 0, :], ohi2, ADD
                )
            nc.sync.dma_start(outr[ic], out_sb)

        emit_attention(0, 0)
        emit_attention(0, 1)
        emit_attention(1, 0)
        emit_epilogue(0)
        emit_attention(1, 1, tail=True)
        emit_epilogue(1, tail=True)


_CACHE = {}


def _get_nc():
    if "nc" not in _CACHE:
        nc = bacc.Bacc("TRN2", debug=False, enable_asserts=False)
        xt = nc.dram_tensor(
            "xt_in", [Q // 512, CC, P, 512], F32R, kind="ExternalInput"
        ).ap()
        ebt = nc.dram_tensor(
            "eb_in", [NI, NJ, P, 512], BF16, kind="ExternalInput"
        ).ap()
        wt = nc.dram_tensor(
            "wt_in", [4, CC, P, EL], F32R, kind="ExternalInput"
        ).ap()
        wot = nc.dram_tensor(
            "wot_in", [P, EC, CQ], F32R, kind="ExternalInput"
        ).ap()
        bg = nc.dram_tensor("bg_in", [P, EC], F32, kind="ExternalInput").ap()
        sel_in = nc.dram_tensor("sel_in", [P, P], F32R, kind="ExternalInput").ap()
        outp = nc.dram_tensor(
            "out", [NI, P, 4, CQ], F32, kind="ExternalOutput"
        ).ap()
        with tile.TileContext(nc) as tc:
            _emit(tc, xt, ebt, wt, wot, bg, sel_in, outp)
        nc.compile()
        _CACHE["nc"] = nc
    return _CACHE["nc"]


LAST_RESULTS = None


def kernel(q_x, kv_x, bias, w_qkv, w_o, b_o, w_g, b_g):
    global LAST_RESULTS
    q_x = np.asarray(q_x, np.float32)
    bias = np.asarray(bias, np.float32)
    w_qkv = np.asarray(w_qkv, np.float32)
    w_o = np.asarray(w_o, np.float32)
    b_o = np.asarray(b_o, np.float32)
    w_g = np.asarray(w_g, np.float32)
    b_g = np.asarray(b_g, np.float32)

    # selection matrix: row 64 ones (reciprocal broadcast); also the warmup
    # stationary (values irrelevant there)
    sel = np.zeros((P, P), np.float32)
    sel[64, :] = 1.0
    in_maps = []
    for core in range(8):
        b, qh, hq = core >> 2, (core >> 1) & 1, core & 1
        i0 = qh * QL
        esl = slice(hq * EL, (hq + 1) * EL)
        xTb = q_x[b].T  # [512, 2048]
        # roll keys so this core's queries are columns 0:QL
        xTp = np.concatenate([xTb[:, i0:], xTb[:, :i0]], axis=1)
        # pre-tile: [j4, c, p, 512] with rows = chans c*128+p
        xtp = np.ascontiguousarray(
            xTp.reshape(CC, P, 4, 512).transpose(2, 0, 1, 3), np.float32
        )
        biasTb = bias[b, 0].T  # [keys, queries]
        ebp = np.exp(
            np.concatenate(
                [biasTb[i0:, i0 : i0 + QL], biasTb[:i0, i0 : i0 + QL]], axis=0
            )
        ).astype(ml_dtypes.bfloat16)
        # pre-tile: [ic, jc, p, qq], keys = jc*128 + p
        ebtp = np.ascontiguousarray(
            ebp.reshape(NJ, P, NI, 512).transpose(2, 0, 1, 3)
        )
        wq = w_qkv[0:CQ][esl] * (1.0 / np.sqrt(D))
        wk = w_qkv[CQ : 2 * CQ][esl]
        wv = w_qkv[2 * CQ : 3 * CQ][esl]
        wg = w_g[esl]
        # pre-tile: [quarter(K,Q,V,G), c, p, e] with rows = chans c*128+p
        wtp = np.ascontiguousarray(
            np.stack(
                [w.T.reshape(CC, P, EL) for w in (wk, wq, wv, wg)], axis=0
            ),
            np.float32,
        )
        # woT pre-tiled: [p, o, c] with e-dim = o*128+p
        woTc = w_o[:, esl].T  # [256, 512]
        wotp = np.ascontiguousarray(
            woTc.reshape(EC, P, CQ).transpose(1, 0, 2), np.float32
        )
        bgc = np.ascontiguousarray(
            (0.5 * b_g[esl]).reshape(EC, P).T, np.float32
        )
        in_maps.append(
            {
                "xt_in": xtp,
                "eb_in": ebtp,
                "wt_in": wtp,
                "wot_in": wotp,
                "bg_in": bgc,
                "sel_in": sel,
            }
        )

    nc = _get_nc()
    res = run_bass_kernel_spmd(nc, in_maps, core_ids=list(range(8)))
    LAST_RESULTS = res

    out = np.zeros((B, Q, CQ), np.float32)
    for core in range(8):
        b, qh = core >> 2, (core >> 1) & 1
        i0 = qh * QL
        # out tensor is [NI, P, 4, CQ]: q row = ic*512 + o*128 + p
        arr = res.results[core]["out"]
        out[b, i0 : i0 + QL] += arr.transpose(0, 2, 1, 3).reshape(QL, CQ)
    out += b_o
    return out


# revision 16
# speedup vs baseline: 1.2184x; 1.2184x over previous
"""Trainium2 Bass kernel for nn_Attention_73289321939579.

Gated attention block (AlphaFold-style):
  qkv = q_x @ w_qkv.T ; q /= sqrt(64)
  scores = q k^T + bias ; attn = softmax(scores, keys)
  o = (attn @ v) * sigmoid(q_x @ w_g.T + b_g)
  out = o @ w_o.T + b_o

Sharding over 8 cores: core = b*4 + qh*2 + hq
  b  = batch (2)            -> data parallel
  qh = query half (2x1024)  -> bias/q sliced, output row-sliced
  hq = head quad (2x4 heads)-> tensor parallel; partial outputs summed on host

v3 structure: ONE tiling-mode switch (mode switches drain the PE array):
  A (128x128): warmup + all projections (v projection emits both pairs
    at N=256). All host inputs are PRE-TILED so every dma_start reads a
    contiguous block (row-strided sources cost per-row descriptors and
    stretched the v2 DMA lead-in to ~25us).
  B (64x128 row-tiled): attention AND epilogue. S^T per head: K=64 head
    dims -> tiles T0/T8 run concurrently (227ns/pair measured). O: K=128
    keys ping-pongs lo/hi halves into separate banks per head
    (same-bank alternating-tile accumulation is FATAL on hw) and is
    summed at block end via DVE copy+add (TensorTensor reads at most
    one PSUM operand). hp-blocked: 2x2 O banks + 2 S slabs = 8 banks.
    The epilogue runs row-tiled too (reciprocal broadcast via a
    selection row on tile T8 over prezeroed fp32r tiles; output
    projection halves summed like O), so epilogue(0) is emitted between
    attention blocks and hides under the exp-bound window.
Elementwise: exp on ACT (the kernel bottleneck: 64 x [128,1024] fp32
psum -> bf16 sbuf, ~68us; the ACT queue carries NOTHING else - in v2
the bias-fetch DIRECT2D descriptors on this queue cost 40us). Bias
multiply on DVE in bf16. exp(bias) is precomputed on host, shipped
bf16 pre-tiled, and kept RESIDENT per query-chunk (2MB) - 8 grouped
sync-queue DMAs total instead of 64 scalar-queue fetches.
"""

import sys

for _p in ("/opt/trn_rl_repo",):
    if _p not in sys.path:
        sys.path.insert(0, _p)

import numpy as np
import ml_dtypes

import concourse.bass as bass  # noqa: F401
import concourse.mybir as mybir
import concourse.tile as tile
from concourse import bacc
from concourse.bass_utils import run_bass_kernel_spmd

# ---- problem dims (hardcoded per contest contract) ----
B, Q, CQ = 2, 2048, 512
H, D = 8, 64
P = 128
QL = 1024          # queries per core
EL = 256           # e-dims per core (4 heads x 64)
HL = 4             # heads per core
CC = CQ // P       # 4 contraction chunks over channels
EC = EL // P       # 2 head-pairs
NJ = Q // P        # 16 key chunks
NI = QL // 512     # 2 query chunks of 512
NG = 4             # eb dma groups per query chunk (4 key-chunks each)

F32 = mybir.dt.float32
F32R = mybir.dt.float32r
BF16 = mybir.dt.bfloat16
MUL = mybir.AluOpType.mult
ADD = mybir.AluOpType.add
EXP = mybir.ActivationFunctionType.Exp
TANH = mybir.ActivationFunctionType.Tanh

# wt_in quarter order (DMA/consumption order): K, Q, V, G
OFF_K, OFF_Q, OFF_V, OFF_G = 0, EL, 2 * EL, 3 * EL
QUARTER_OFF = [OFF_K, OFF_Q, OFF_V, OFF_G]


def _r(ap):
    """float32r view for matmul operands (single-pass fp22 on the PE)."""
    return ap.bitcast(F32R)


def _emit(tc, xt, ebt, wt, wot, bg, sel_in, outp):
    nc = tc.nc
    from contextlib import ExitStack

    with ExitStack() as ctx:
        const = ctx.enter_context(tc.tile_pool(name="const", bufs=1))
        biasp = ctx.enter_context(tc.tile_pool(name="biasp", bufs=2))
        # xT/wT (phase A only) share this pool with the deep es/pt rings
        # (phase B only) so the allocator reuses the same SBUF bytes
        bigp = ctx.enter_context(tc.tile_pool(name="bigp", bufs=1))
        workp = ctx.enter_context(tc.tile_pool(name="workp", bufs=2))
        odp = ctx.enter_context(tc.tile_pool(name="odp", bufs=4))
        psum = ctx.enter_context(tc.tile_pool(name="psum", bufs=2, space="PSUM"))

        # ---- small constants FIRST so compute can start immediately ----
        sel_sb = const.tile([P, P], F32R, name="sel_sb", tag="sel_sb")
        nc.sync.dma_start(sel_sb, sel_in)
        bg_sb = const.tile([P, EC], F32, name="bg_sb", tag="bg_sb")
        nc.sync.dma_start(bg_sb, bg)
        woT_sb = const.tile([P, EC, CQ], F32R, name="woT_sb", tag="woT_sb")
        nc.sync.dma_start(woT_sb, wot)

        # ---- bulk inputs; every source block is contiguous in DRAM, in
        # consumption order: wk, xt[0], wv, wq, wg, xt[1..3] ----
        wT_sb = bigp.tile([P, CC, 4 * EL], F32R, name="wT_sb", tag="wT_sb")
        xT_sb = bigp.tile([P, CC, Q], F32R, name="xT_sb", tag="xT_sb")

        def dma_w(q4):
            off = QUARTER_OFF[q4]
            for c in range(CC):
                nc.sync.dma_start(wT_sb[:, c, off : off + EL], wt[q4, c])

        def dma_x(j4):
            for c in range(CC):
                nc.sync.dma_start(
                    xT_sb[:, c, j4 * 512 : (j4 + 1) * 512], xt[j4, c]
                )

        dma_w(0)      # wk
        dma_x(0)
        dma_w(2)      # wv
        dma_w(1)      # wq
        dma_w(3)      # wg
        for j4 in range(1, Q // 512):
            dma_x(j4)

        # ---- resident intermediates ----
        kT_sb = const.tile([P, EC, Q], F32R, name="kT_sb", tag="kT_sb")
        q_sb = const.tile([P, EC, QL], F32R, name="q_sb", tag="q_sb")
        gp_sb = const.tile([P, EC, QL], F32, name="gp_sb", tag="gp_sb")
        og_sb = const.tile([P, EC, QL], F32R, name="og_sb", tag="og_sb")
        # V augmented with a ones column: [keychunk-part, jc, head, 65], bf16
        va_sb = const.tile([P, NJ, HL, D + 1], BF16, name="va_sb", tag="va_sb")
        nc.vector.memset(va_sb[:, :, :, D], 1.0)
        # prezeroed fp32r reciprocal tiles: the ACT rounding copy writes rows
        # 0:65 only; rows 65:128 stay zero so the row-tiled broadcast matmul
        # (T8, K=64) sees zeros outside the selection row
        recr_sbs = []
        for ri in range(2):
            rr = const.tile([P, 512], F32R, name=f"recr{ri}", tag=f"recr{ri}")
            nc.vector.memset(rr.bitcast(F32), 0.0)
            recr_sbs.append(rr)

        # ================= phase A: 128x128 =================
        # warmup burst: covers the pre-tiled DMA lead-in (~5us) + HAM ramp
        warm_ps = psum.tile([P, 2, 512], F32, tag="s", name="warm_ps")
        for wi in range(30):
            nc.tensor.matmul(
                warm_ps[:, 0, 0:P], _r(sel_sb), _r(sel_sb),
                start=(wi == 0), stop=(wi == 29),
            )
        warm_sb = workp.tile([P, P], F32, name="warm_sb", tag="recf")
        nc.vector.tensor_copy(out=warm_sb[:, 0:P], in_=warm_ps[:, 0, 0:P])

        def emit_k_pair(ec, j4):
            if True:
                sl = slice(j4 * 512, (j4 + 1) * 512)
                ps_k = psum.tile([P, 2, 512], F32, tag="s", name="ps_k")
                for c in range(CC):
                    nc.tensor.matmul(
                        ps_k[:, 0, :],
                        _r(wT_sb[:, c, OFF_K + ec * P : OFF_K + (ec + 1) * P]),
                        _r(xT_sb[:, c, sl]),
                        start=(c == 0), stop=(c == CC - 1),
                    )
                nc.vector.tensor_copy(out=kT_sb[:, ec, sl], in_=ps_k[:, 0, :])

        def emit_q_pair(ec, icc):
            if True:
                sl = slice(icc * 512, (icc + 1) * 512)
                ps_q = psum.tile([P, 2, 512], F32, tag="s", name="ps_q")
                for c in range(CC):
                    nc.tensor.matmul(
                        ps_q[:, 0, :],
                        _r(wT_sb[:, c, OFF_Q + ec * P : OFF_Q + (ec + 1) * P]),
                        _r(xT_sb[:, c, sl]),
                        start=(c == 0), stop=(c == CC - 1),
                    )
                nc.vector.tensor_copy(out=q_sb[:, ec, sl], in_=ps_q[:, 0, :])

        def emit_v_all(j4):
            # both pairs at once: out [keys 128, 256 e-dims] per key chunk
            for jc in range(4 * j4, 4 * j4 + 4):
                ps_v = psum.tile([P, 2, 512], F32, tag="s", name="ps_v")
                for c in range(CC):
                    nc.tensor.matmul(
                        ps_v[:, 0, 0:EL],
                        _r(xT_sb[:, c, jc * P : (jc + 1) * P]),
                        _r(wT_sb[:, c, OFF_V : OFF_V + EL]),
                        start=(c == 0), stop=(c == CC - 1),
                    )
                nc.vector.tensor_copy(
                    out=va_sb[:, jc, :, 0:D],
                    in_=ps_v[:, 0, 0:EL].rearrange("p (h d) -> p h d", h=HL),
                )

        def emit_gate_pair(ec, icc):
            # sigmoid(x wg^T + bg) = 0.5*tanh(0.5 x wg^T + 0.5 bg) + 0.5
            # (Tanh shares the ACT table set with Exp -> no table swap)
            if True:
                sl = slice(icc * 512, (icc + 1) * 512)
                ps_g = psum.tile([P, 2, 512], F32, tag="s", name="ps_g")
                for c in range(CC):
                    nc.tensor.matmul(
                        ps_g[:, 0, :],
                        _r(wT_sb[:, c, OFF_G + ec * P : OFF_G + (ec + 1) * P]),
                        _r(xT_sb[:, c, sl]),
                        start=(c == 0), stop=(c == CC - 1),
                    )
                nc.scalar.activation(
                    gp_sb[:, ec, sl], ps_g[:, 0, :], TANH,
                    bias=bg_sb[:, ec : ec + 1], scale=0.5,
                )

        # j4-interleaved so the PE chases the xt chunk uploads without
        # idling (an idle gap over the HAM window re-throttles the clock
        # and the whole projection phase then runs at 1.2GHz)
        for j4 in range(Q // 512):
            emit_k_pair(0, j4)
            emit_k_pair(1, j4)
            emit_v_all(j4)
            if j4 < NI:
                emit_q_pair(0, j4)
                emit_q_pair(1, j4)
                emit_gate_pair(0, j4)
                emit_gate_pair(1, j4)
        nc.vector.tensor_scalar(gp_sb, gp_sb, 0.5, 0.5, MUL, ADD)

        # ================= phase B: 64x128 row-tiled =================
        o_sb_all = {}  # (ic, h) -> [65, 512] f32 numerator + denominator row
        eb_tiles = {}

        def emit_eb_fetch(ic):
            ebt_sb = biasp.tile([P, NJ, 512], BF16, name=f"eb{ic}", tag="eb")
            for g in range(NG):
                nc.sync.dma_start(ebt_sb[:, 4 * g : 4 * g + 4, :], ebt[ic, g])
            eb_tiles[ic] = ebt_sb

        def emit_attention(ic, hp, tail=False, interleave=None):
            """16 key chunks for query chunk ic, head pair hp (heads 2hp,2hp+1)."""
            isl = slice(ic * 512, (ic + 1) * 512)
            ebt_sb = eb_tiles[ic]
            o_ps = [
                psum.tile([P, 512], F32, tag="o", name=f"o_ps{ic}_{hp}_{i}", bufs=4)
                for i in range(4)
            ]  # [h_even lo, h_even hi, h_odd lo, h_odd hi]
            for jc in range(NJ):
                # epilogue pieces for the previous query chunk drip in here
                # so their DVE work never forms a burst ahead of this block's
                # multiplies in the strict-FIFO DVE queue
                if interleave is not None and jc % 4 == 3:
                    interleave((jc - 3) // 4)
                s_ps = psum.tile([P, 2, 512], F32, tag="s", name="s_ps")
                nc.tensor.matmul(
                    s_ps[:, 0, :],
                    _r(kT_sb[0:64, hp, jc * P : (jc + 1) * P]),
                    _r(q_sb[0:64, hp, isl]),
                    start=True, stop=True,
                )
                nc.tensor.matmul(
                    s_ps[:, 1, :],
                    _r(kT_sb[64:128, hp, jc * P : (jc + 1) * P]),
                    _r(q_sb[64:128, hp, isl]),
                    start=True, stop=True,
                )
                es_sb = bigp.tile([P, 2, 512], BF16, name="es_sb", tag="es", bufs=3)
                nc.scalar.activation(
                    es_sb.rearrange("p a b -> p (a b)"),
                    s_ps.rearrange("p a b -> p (a b)"), EXP,
                )
                pt_sb = bigp.tile([P, 2, 512], BF16, name="pt_sb", tag="pt", bufs=3)
                ebb = ebt_sb[:, jc, None, :].to_broadcast([P, 2, 512])
                nc.vector.tensor_tensor(pt_sb, es_sb, ebb, MUL)
                for hh in range(2):
                    h = 2 * hp + hh
                    nc.tensor.matmul(
                        o_ps[2 * hh + 0][0:65, :],
                        va_sb[0:64, jc, h, :],
                        pt_sb[0:64, hh, :],
                        start=(jc == 0), stop=(jc == NJ - 1),
                    )
                    nc.tensor.matmul(
                        o_ps[2 * hh + 1][0:65, :],
                        va_sb[64:128, jc, h, :],
                        pt_sb[64:128, hh, :],
                        start=(jc == 0), stop=(jc == NJ - 1),
                    )
            for hh in range(2):
                h = 2 * hp + hh
                # lo + hi halves: TensorTensor reads at most one psum operand,
                # so stage the hi bank through SBUF (DVE; ACT stays exp-only)
                ohi = workp.tile([65, 512], F32, name="ohi", tag="ohi")
                if tail:
                    nc.scalar.copy(ohi, o_ps[2 * hh + 1][0:65, :])
                else:
                    nc.vector.tensor_copy(out=ohi, in_=o_ps[2 * hh + 1][0:65, :])
                # 6 live max: 4 from chunk 0 + 2 from attention(1,0); the
                # interleaved epilogue(0) frees chunk 0's before (1,1) drains
                osb = odp.tile([65, 512], F32, name=f"o_sb{ic}_{h}", tag="od",
                               bufs=6)
                nc.vector.tensor_tensor(
                    osb, o_ps[2 * hh + 0][0:65, :], ohi, ADD
                )
                o_sb_all[(ic, h)] = osb

        outr = outp  # [NI, P, 4, CQ] pre-tiled output

        epi_stg = {}

        def emit_epilogue_head(ic, h):
            """normalize one head of query chunk ic (row-tiled)."""
            if ic not in epi_stg:
                epi_stg[ic] = (
                    workp.tile([64, EC, 512], F32, name="og_stg", tag="ogstg"),
                    workp.tile([P, EC, 512], F32, name="ocp_sb", tag="ocp"),
                )
            og_stg, ocp_sb = epi_stg[ic]
            if True:
                hp, hh = h // 2, h % 2
                o_sb = o_sb_all[(ic, h)]
                recf_sb = workp.tile([P, 512], F32, name="recf_sb", tag="recf")
                # approx recip over the whole [65,512] accumulator (single-row
                # slices miscompute in the custom-DVE path); row 64 holds the
                # softmax denominators
                nc.vector.reciprocal_approx_fast(out=recf_sb[0:65, :], in_=o_sb)
                # fp32r rounding for the matmul operand (BIR requirement)
                recr_sb = recr_sbs[h % 2]
                nc.vector.tensor_copy(out=recr_sb[0:65, :], in_=recf_sb[0:65, :])
                # broadcast row 64 to all partitions: T8 matmul, stationary =
                # sel rows 64:128 (indicator at relative row 0, zeros below),
                # moving = recr rows 64:128 (rec at relative row 0, prezeroed
                # below)
                bc_ps = psum.tile([P, 2, 512], F32, tag="s", name="bc_ps")
                nc.tensor.matmul(
                    bc_ps[:, 0, :], _r(sel_sb[64:128, :]), recr_sb[64:128, :],
                    start=True, stop=True,
                )
                oc_dst = ocp_sb[0:64, hp, :] if hh == 0 else og_stg[:, hp, :]
                nc.vector.tensor_tensor(
                    oc_dst, bc_ps[0:64, 0, :], o_sb[0:64, :], MUL
                )

        def emit_epilogue_finish(ic, tail=False):
            """gate + output projection for query chunk ic."""
            isl = slice(ic * 512, (ic + 1) * 512)
            og_stg, ocp_sb = epi_stg[ic]
            nc.sync.dma_start(ocp_sb[64:128, :, :], og_stg)
            for hp in range(EC):
                nc.vector.tensor_tensor(
                    og_sb[:, hp, isl], ocp_sb[:, hp, :], gp_sb[:, hp, isl], MUL
                )
            # output projection: K=256 split into 64-row quarters; T0 chains
            # the lo halves, T8 the hi halves, DVE sums the two banks
            out_sb = odp.tile([P, 4, CQ], F32, name="out_sb", tag="outsb", bufs=2)
            for ip4 in range(4):
                ip = ic * 4 + ip4
                slab = psum.tile([P, 2, 512], F32, tag="s", name="ps_o")
                for ec in range(EC):
                    nc.tensor.matmul(
                        slab[:, 0, :],
                        _r(og_sb[0:64, ec, ip * P : (ip + 1) * P]),
                        _r(woT_sb[0:64, ec, :]),
                        start=(ec == 0), stop=(ec == EC - 1),
                    )
                    nc.tensor.matmul(
                        slab[:, 1, :],
                        _r(og_sb[64:128, ec, ip * P : (ip + 1) * P]),
                        _r(woT_sb[64:128, ec, :]),
                        start=(ec == 0), stop=(ec == EC - 1),
                    )
                ohi2 = workp.tile([P, 512], F32, name="ohi2", tag="ohi2")
                if tail:
                    nc.scalar.copy(ohi2, slab[:, 1, :])
                else:
                    nc.vector.tensor_copy(out=ohi2, in_=slab[:, 1, :])
                nc.vector.tensor_tensor(
                    out_sb[:, ip4, :], slab[:, 0, :], ohi2, ADD
                )
            nc.sync.dma_start(outr[ic], out_sb)

        emit_eb_fetch(0)
        emit_attention(0, 0)
        emit_eb_fetch(1)
        emit_attention(0, 1)
        emit_attention(1, 0)
        emit_attention(
            1, 1, tail=True,
            interleave=lambda i: emit_epilogue_head(0, i),
        )
        emit_epilogue_finish(0)
        for h in range(HL):
            emit_epilogue_head(1, h)
        emit_epilogue_finish(1, tail=True)


_CACHE = {}


def _get_nc():
    if "nc" not in _CACHE:
        nc = bacc.Bacc("TRN2", debug=False, enable_asserts=False)
        xt = nc.dram_tensor(
            "xt_in", [Q // 512, CC, P, 512], F32R, kind="ExternalInput"
        ).ap()
        ebt = nc.dram_tensor(
            "eb_in", [NI, NG, P, 4 * 512], BF16, kind="ExternalInput"
        ).ap()
        wt = nc.dram_tensor(
            "wt_in", [4, CC, P, EL], F32R, kind="ExternalInput"
        ).ap()
        wot = nc.dram_tensor(
            "wot_in", [P, EC, CQ], F32R, kind="ExternalInput"
        ).ap()
        bg = nc.dram_tensor("bg_in", [P, EC], F32, kind="ExternalInput").ap()
        sel_in = nc.dram_tensor("sel_in", [P, P], F32R, kind="ExternalInput").ap()
        outp = nc.dram_tensor(
            "out", [NI, P, 4, CQ], F32, kind="ExternalOutput"
        ).ap()
        with tile.TileContext(nc) as tc:
            _emit(tc, xt, ebt, wt, wot, bg, sel_in, outp)
        nc.compile()
        _CACHE["nc"] = nc
    return _CACHE["nc"]


LAST_RESULTS = None


def kernel(q_x, kv_x, bias, w_qkv, w_o, b_o, w_g, b_g):
    global LAST_RESULTS
    q_x = np.asarray(q_x, np.float32)
    bias = np.asarray(bias, np.float32)
    w_qkv = np.asarray(w_qkv, np.float32)
    w_o = np.asarray(w_o, np.float32)
    b_o = np.asarray(b_o, np.float32)
    w_g = np.asarray(w_g, np.float32)
    b_g = np.asarray(b_g, np.float32)

    # selection matrix: row 64 ones (reciprocal broadcast); also the warmup
    # stationary (values irrelevant there)
    sel = np.zeros((P, P), np.float32)
    sel[64, :] = 1.0
    in_maps = []
    for core in range(8):
        b, qh, hq = core >> 2, (core >> 1) & 1, core & 1
        i0 = qh * QL
        esl = slice(hq * EL, (hq + 1) * EL)
        xTb = q_x[b].T  # [512, 2048]
        # roll keys so this core's queries are columns 0:QL
        xTp = np.concatenate([xTb[:, i0:], xTb[:, :i0]], axis=1)
        # pre-tile: [j4, c, p, 512] with rows = chans c*128+p
        xtp = np.ascontiguousarray(
            xTp.reshape(CC, P, 4, 512).transpose(2, 0, 1, 3), np.float32
        )
        biasTb = bias[b, 0].T  # [keys, queries]
        ebp = np.exp(
            np.concatenate(
                [biasTb[i0:, i0 : i0 + QL], biasTb[:i0, i0 : i0 + QL]], axis=0
            )
        ).astype(ml_dtypes.bfloat16)
        # pre-tile: [ic, g, p, jj*512+qq], keys = (4g+jj)*128 + p
        ebtp = np.ascontiguousarray(
            ebp.reshape(NG, 4, P, NI, 512)
            .transpose(3, 0, 2, 1, 4)
            .reshape(NI, NG, P, 4 * 512)
        )
        wq = w_qkv[0:CQ][esl] * (1.0 / np.sqrt(D))
        wk = w_qkv[CQ : 2 * CQ][esl]
        wv = w_qkv[2 * CQ : 3 * CQ][esl]
        wg = w_g[esl]
        # pre-tile: [quarter(K,Q,V,G), c, p, e] with rows = chans c*128+p
        wtp = np.ascontiguousarray(
            np.stack(
                [w.T.reshape(CC, P, EL) for w in (wk, wq, wv, wg)], axis=0
            ),
            np.float32,
        )
        # woT pre-tiled: [p, o, c] with e-dim = o*128+p
        woTc = w_o[:, esl].T  # [256, 512]
        wotp = np.ascontiguousarray(
            woTc.reshape(EC, P, CQ).transpose(1, 0, 2), np.float32
        )
        bgc = np.ascontiguousarray(
            (0.5 * b_g[esl]).reshape(EC, P).T, np.float32
        )
        in_maps.append(
            {
                "xt_in": xtp,
                "eb_in": ebtp,
                "wt_in": wtp,
                "wot_in": wotp,
                "bg_in": bgc,
                "sel_in": sel,
            }
        )

    nc = _get_nc()
    res = run_bass_kernel_spmd(nc, in_maps, core_ids=list(range(8)))
    LAST_RESULTS = res

    out = np.zeros((B, Q, CQ), np.float32)
    for core in range(8):
        b, qh = core >> 2, (core >> 1) & 1
        i0 = qh * QL
        # out tensor is [NI, P, 4, CQ]: q row = ic*512 + o*128 + p
        arr = res.results[core]["out"]
        out[b, i0 : i0 + QL] += arr.transpose(0, 2, 1, 3).reshape(QL, CQ)
    out += b_o
    return out


# revision 18
# speedup vs baseline: 1.2186x; 1.0001x over previous
"""Trainium2 Bass kernel for nn_Attention_73289321939579.

Gated attention block (AlphaFold-style):
  qkv = q_x @ w_qkv.T ; q /= sqrt(64)
  scores = q k^T + bias ; attn = softmax(scores, keys)
  o = (attn @ v) * sigmoid(q_x @ w_g.T + b_g)
  out = o @ w_o.T + b_o

Sharding over 8 cores: core = b*4 + qh*2 + hq
  b  = batch (2)            -> data parallel
  qh = query half (2x1024)  -> bias/q sliced, output row-sliced
  hq = head quad (2x4 heads)-> tensor parallel; partial outputs summed on host

v3 structure: ONE tiling-mode switch (mode switches drain the PE array):
  A (128x128): warmup + all projections (v projection emits both pairs
    at N=256). All host inputs are PRE-TILED so every dma_start reads a
    contiguous block (row-strided sources cost per-row descriptors and
    stretched the v2 DMA lead-in to ~25us).
  B (64x128 row-tiled): attention AND epilogue. S^T per head: K=64 head
    dims -> tiles T0/T8 run concurrently (227ns/pair measured). O: K=128
    keys ping-pongs lo/hi halves into separate banks per head
    (same-bank alternating-tile accumulation is FATAL on hw) and is
    summed at block end via DVE copy+add (TensorTensor reads at most
    one PSUM operand). hp-blocked: 2x2 O banks + 2 S slabs = 8 banks.
    The epilogue runs row-tiled too (reciprocal broadcast via a
    selection row on tile T8 over prezeroed fp32r tiles; output
    projection halves summed like O), so epilogue(0) is emitted between
    attention blocks and hides under the exp-bound window.
Elementwise: exp on ACT (the kernel bottleneck: 64 x [128,1024] fp32
psum -> bf16 sbuf, ~68us; the ACT queue carries NOTHING else - in v2
the bias-fetch DIRECT2D descriptors on this queue cost 40us). Bias
multiply on DVE in bf16. exp(bias) is precomputed on host, shipped
bf16 pre-tiled, and kept RESIDENT per query-chunk (2MB) - 8 grouped
sync-queue DMAs total instead of 64 scalar-queue fetches.
"""

import sys

for _p in ("/opt/trn_rl_repo",):
    if _p not in sys.path:
        sys.path.insert(0, _p)

import numpy as np
import ml_dtypes

import concourse.bass as bass  # noqa: F401
import concourse.mybir as mybir
import concourse.tile as tile
from concourse import bacc
from concourse.bass_utils import run_bass_kernel_spmd

# ---- problem dims (hardcoded per contest contract) ----
B, Q, CQ = 2, 2048, 512
H, D = 8, 64
P = 128
QL = 1024          # queries per core
EL = 256           # e-dims per core (4 heads x 64)
HL = 4             # heads per core
CC = CQ // P       # 4 contraction chunks over channels
EC = EL // P       # 2 head-pairs
NJ = Q // P        # 16 key chunks
NI = QL // 512     # 2 query chunks of 512
NG = 4             # eb dma groups per query chunk (4 key-chunks each)

F32 = mybir.dt.float32
F32R = mybir.dt.float32r
BF16 = mybir.dt.bfloat16
MUL = mybir.AluOpType.mult
ADD = mybir.AluOpType.add
EXP = mybir.ActivationFunctionType.Exp
TANH = mybir.ActivationFunctionType.Tanh

# wt_in quarter order (DMA/consumption order): K, Q, V, G
OFF_K, OFF_Q, OFF_V, OFF_G = 0, EL, 2 * EL, 3 * EL
QUARTER_OFF = [OFF_K, OFF_Q, OFF_V, OFF_G]


def _r(ap):
    """float32r view for matmul operands (single-pass fp22 on the PE)."""
    return ap.bitcast(F32R)


def _emit(tc, xt, ebt, wt, wot, bg, sel_in, outp):
    nc = tc.nc
    from contextlib import ExitStack

    with ExitStack() as ctx:
        const = ctx.enter_context(tc.tile_pool(name="const", bufs=1))
        biasp = ctx.enter_context(tc.tile_pool(name="biasp", bufs=2))
        # xT/wT (phase A only) share this pool with the deep es/pt rings
        # (phase B only) so the allocator reuses the same SBUF bytes
        bigp = ctx.enter_context(tc.tile_pool(name="bigp", bufs=1))
        workp = ctx.enter_context(tc.tile_pool(name="workp", bufs=2))
        odp = ctx.enter_context(tc.tile_pool(name="odp", bufs=4))
        psum = ctx.enter_context(tc.tile_pool(name="psum", bufs=2, space="PSUM"))

        # ---- small constants FIRST so compute can start immediately ----
        sel_sb = const.tile([P, P], F32R, name="sel_sb", tag="sel_sb")
        nc.sync.dma_start(sel_sb, sel_in)
        bg_sb = const.tile([P, EC], F32, name="bg_sb", tag="bg_sb")
        nc.sync.dma_start(bg_sb, bg)
        woT_sb = const.tile([P, EC, CQ], F32R, name="woT_sb", tag="woT_sb")
        nc.sync.dma_start(woT_sb, wot)

        # ---- bulk inputs; every source block is contiguous in DRAM, in
        # consumption order: wk, xt[0], wv, wq, wg, xt[1..3] ----
        wT_sb = bigp.tile([P, CC, 4 * EL], F32R, name="wT_sb", tag="wT_sb")
        xT_sb = bigp.tile([P, CC, Q], F32R, name="xT_sb", tag="xT_sb")

        def dma_w(q4):
            off = QUARTER_OFF[q4]
            for c in range(CC):
                nc.sync.dma_start(wT_sb[:, c, off : off + EL], wt[q4, c])

        def dma_x(j4):
            for c in range(CC):
                nc.sync.dma_start(
                    xT_sb[:, c, j4 * 512 : (j4 + 1) * 512], xt[j4, c]
                )

        dma_w(0)      # wk
        dma_x(0)
        dma_w(2)      # wv
        dma_w(1)      # wq
        dma_w(3)      # wg
        for j4 in range(1, Q // 512):
            dma_x(j4)

        # ---- resident intermediates ----
        kT_sb = const.tile([P, EC, Q], F32R, name="kT_sb", tag="kT_sb")
        q_sb = const.tile([P, EC, QL], F32R, name="q_sb", tag="q_sb")
        gp_sb = const.tile([P, EC, QL], F32, name="gp_sb", tag="gp_sb")
        og_sb = const.tile([P, EC, QL], F32R, name="og_sb", tag="og_sb")
        # V augmented with a ones column: [keychunk-part, jc, head, 65], bf16
        va_sb = const.tile([P, NJ, HL, D + 1], BF16, name="va_sb", tag="va_sb")
        nc.vector.memset(va_sb[:, :, :, D], 1.0)
        # prezeroed fp32r reciprocal tiles: the ACT rounding copy writes rows
        # 0:65 only; rows 65:128 stay zero so the row-tiled broadcast matmul
        # (T8, K=64) sees zeros outside the selection row
        recr_sbs = []
        for ri in range(2):
            rr = const.tile([P, 512], F32R, name=f"recr{ri}", tag=f"recr{ri}")
            nc.vector.memset(rr.bitcast(F32), 0.0)
            recr_sbs.append(rr)

        # ================= phase A: 128x128 =================
        # warmup burst: covers the pre-tiled DMA lead-in and ramps HAM.
        # N=512 streams (vs N=128) keep the PE duty cycle near 100% so the
        # activity monitor actually flips to full clock
        warm_ps = psum.tile([P, 2, 512], F32, tag="s", name="warm_ps")
        for wi in range(14):
            nc.tensor.matmul(
                warm_ps[:, 0, :], _r(sel_sb), woT_sb[:, 0, :],
                start=(wi == 0), stop=(wi == 13),
            )
        warm_sb = workp.tile([P, 512], F32, name="warm_sb", tag="recf")
        nc.vector.tensor_copy(out=warm_sb, in_=warm_ps[:, 0, :])

        def emit_k_pair(ec, j4):
            if True:
                sl = slice(j4 * 512, (j4 + 1) * 512)
                ps_k = psum.tile([P, 2, 512], F32, tag="s", name="ps_k")
                for c in range(CC):
                    nc.tensor.matmul(
                        ps_k[:, 0, :],
                        _r(wT_sb[:, c, OFF_K + ec * P : OFF_K + (ec + 1) * P]),
                        _r(xT_sb[:, c, sl]),
                        start=(c == 0), stop=(c == CC - 1),
                    )
                nc.vector.tensor_copy(out=kT_sb[:, ec, sl], in_=ps_k[:, 0, :])

        def emit_q_pair(ec, icc):
            if True:
                sl = slice(icc * 512, (icc + 1) * 512)
                ps_q = psum.tile([P, 2, 512], F32, tag="s", name="ps_q")
                for c in range(CC):
                    nc.tensor.matmul(
                        ps_q[:, 0, :],
                        _r(wT_sb[:, c, OFF_Q + ec * P : OFF_Q + (ec + 1) * P]),
                        _r(xT_sb[:, c, sl]),
                        start=(c == 0), stop=(c == CC - 1),
                    )
                nc.vector.tensor_copy(out=q_sb[:, ec, sl], in_=ps_q[:, 0, :])

        def emit_v_all(j4):
            # both pairs at once: out [keys 128, 256 e-dims] per key chunk
            for jc in range(4 * j4, 4 * j4 + 4):
                ps_v = psum.tile([P, 2, 512], F32, tag="s", name="ps_v")
                for c in range(CC):
                    nc.tensor.matmul(
                        ps_v[:, 0, 0:EL],
                        _r(xT_sb[:, c, jc * P : (jc + 1) * P]),
                        _r(wT_sb[:, c, OFF_V : OFF_V + EL]),
                        start=(c == 0), stop=(c == CC - 1),
                    )
                nc.vector.tensor_copy(
                    out=va_sb[:, jc, :, 0:D],
                    in_=ps_v[:, 0, 0:EL].rearrange("p (h d) -> p h d", h=HL),
                )

        def emit_gate_pair(ec, icc):
            # sigmoid(x wg^T + bg) = 0.5*tanh(0.5 x wg^T + 0.5 bg) + 0.5
            # (Tanh shares the ACT table set with Exp -> no table swap)
            if True:
                sl = slice(icc * 512, (icc + 1) * 512)
                ps_g = psum.tile([P, 2, 512], F32, tag="s", name="ps_g")
                for c in range(CC):
                    nc.tensor.matmul(
                        ps_g[:, 0, :],
                        _r(wT_sb[:, c, OFF_G + ec * P : OFF_G + (ec + 1) * P]),
                        _r(xT_sb[:, c, sl]),
                        start=(c == 0), stop=(c == CC - 1),
                    )
                nc.scalar.activation(
                    gp_sb[:, ec, sl], ps_g[:, 0, :], TANH,
                    bias=bg_sb[:, ec : ec + 1], scale=0.5,
                )

        # j4-interleaved so the PE chases the xt chunk uploads without
        # idling (an idle gap over the HAM window re-throttles the clock
        # and the whole projection phase then runs at 1.2GHz)
        for j4 in range(Q // 512):
            emit_k_pair(0, j4)
            emit_k_pair(1, j4)
            emit_v_all(j4)
            if j4 < NI:
                emit_q_pair(0, j4)
                emit_q_pair(1, j4)
                emit_gate_pair(0, j4)
                emit_gate_pair(1, j4)
        nc.vector.tensor_scalar(gp_sb, gp_sb, 0.5, 0.5, MUL, ADD)

        # ================= phase B: 64x128 row-tiled =================
        o_sb_all = {}  # (ic, h) -> [65, 512] f32 numerator + denominator row
        eb_tiles = {}

        def emit_eb_fetch(ic):
            ebt_sb = biasp.tile([P, NJ, 512], BF16, name=f"eb{ic}", tag="eb")
            for g in range(NG):
                nc.sync.dma_start(ebt_sb[:, 4 * g : 4 * g + 4, :], ebt[ic, g])
            eb_tiles[ic] = ebt_sb

        def emit_attention(ic, hp, tail=False, interleave=None):
            """16 key chunks for query chunk ic, head pair hp (heads 2hp,2hp+1)."""
            isl = slice(ic * 512, (ic + 1) * 512)
            ebt_sb = eb_tiles[ic]
            o_ps = [
                psum.tile([P, 512], F32, tag="o", name=f"o_ps{ic}_{hp}_{i}", bufs=4)
                for i in range(4)
            ]  # [h_even lo, h_even hi, h_odd lo, h_odd hi]
            for jc in range(NJ):
                # epilogue pieces for the previous query chunk drip in here
                # so their DVE work never forms a burst ahead of this block's
                # multiplies in the strict-FIFO DVE queue
                if interleave is not None and jc < len(interleave):
                    interleave[jc]()
                s_ps = psum.tile([P, 2, 512], F32, tag="s", name="s_ps")
                nc.tensor.matmul(
                    s_ps[:, 0, :],
                    _r(kT_sb[0:64, hp, jc * P : (jc + 1) * P]),
                    _r(q_sb[0:64, hp, isl]),
                    start=True, stop=True,
                )
                nc.tensor.matmul(
                    s_ps[:, 1, :],
                    _r(kT_sb[64:128, hp, jc * P : (jc + 1) * P]),
                    _r(q_sb[64:128, hp, isl]),
                    start=True, stop=True,
                )
                es_sb = bigp.tile([P, 2, 512], BF16, name="es_sb", tag="es", bufs=4)
                nc.scalar.activation(
                    es_sb.rearrange("p a b -> p (a b)"),
                    s_ps.rearrange("p a b -> p (a b)"), EXP,
                )
                pt_sb = bigp.tile([P, 2, 512], BF16, name="pt_sb", tag="pt", bufs=3)
                ebb = ebt_sb[:, jc, None, :].to_broadcast([P, 2, 512])
                nc.vector.tensor_tensor(pt_sb, es_sb, ebb, MUL)
                for hh in range(2):
                    h = 2 * hp + hh
                    nc.tensor.matmul(
                        o_ps[2 * hh + 0][0:65, :],
                        va_sb[0:64, jc, h, :],
                        pt_sb[0:64, hh, :],
                        start=(jc == 0), stop=(jc == NJ - 1),
                    )
                    nc.tensor.matmul(
                        o_ps[2 * hh + 1][0:65, :],
                        va_sb[64:128, jc, h, :],
                        pt_sb[64:128, hh, :],
                        start=(jc == 0), stop=(jc == NJ - 1),
                    )
            for hh in range(2):
                h = 2 * hp + hh
                # lo + hi halves: TensorTensor reads at most one psum operand,
                # so stage the hi bank through SBUF (DVE; ACT stays exp-only)
                ohi = workp.tile([65, 512], F32, name="ohi", tag="ohi")
                if tail:
                    nc.scalar.copy(ohi, o_ps[2 * hh + 1][0:65, :])
                else:
                    nc.vector.tensor_copy(out=ohi, in_=o_ps[2 * hh + 1][0:65, :])
                # 6 live max: 4 from chunk 0 + 2 from attention(1,0); the
                # interleaved epilogue(0) frees chunk 0's before (1,1) drains
                osb = odp.tile([65, 512], F32, name=f"o_sb{ic}_{h}", tag="od",
                               bufs=6)
                nc.vector.tensor_tensor(
                    osb, o_ps[2 * hh + 0][0:65, :], ohi, ADD
                )
                o_sb_all[(ic, h)] = osb

        outr = outp  # [NI, P, 4, CQ] pre-tiled output

        epi_stg = {}

        def emit_epilogue_head(ic, h):
            """normalize one head of query chunk ic (row-tiled)."""
            if ic not in epi_stg:
                epi_stg[ic] = (
                    workp.tile([64, EC, 512], F32, name="og_stg", tag="ogstg"),
                    workp.tile([P, EC, 512], F32, name="ocp_sb", tag="ocp"),
                )
            og_stg, ocp_sb = epi_stg[ic]
            if True:
                hp, hh = h // 2, h % 2
                o_sb = o_sb_all[(ic, h)]
                recf_sb = workp.tile([P, 512], F32, name="recf_sb", tag="recf")
                # approx recip over the whole [65,512] accumulator (single-row
                # slices miscompute in the custom-DVE path); row 64 holds the
                # softmax denominators
                nc.vector.reciprocal_approx_fast(out=recf_sb[0:65, :], in_=o_sb)
                # fp32r rounding for the matmul operand (BIR requirement)
                recr_sb = recr_sbs[h % 2]
                nc.vector.tensor_copy(out=recr_sb[0:65, :], in_=recf_sb[0:65, :])
                # broadcast row 64 to all partitions: T8 matmul, stationary =
                # sel rows 64:128 (indicator at relative row 0, zeros below),
                # moving = recr rows 64:128 (rec at relative row 0, prezeroed
                # below)
                bc_ps = psum.tile([P, 2, 512], F32, tag="s", name="bc_ps")
                nc.tensor.matmul(
                    bc_ps[:, 0, :], _r(sel_sb[64:128, :]), recr_sb[64:128, :],
                    start=True, stop=True,
                )
                oc_dst = ocp_sb[0:64, hp, :] if hh == 0 else og_stg[:, hp, :]
                nc.vector.tensor_tensor(
                    oc_dst, bc_ps[0:64, 0, :], o_sb[0:64, :], MUL
                )

        out_sbs = {}

        def emit_epilogue_og(ic):
            """relocate odd heads + apply the gate for query chunk ic."""
            isl = slice(ic * 512, (ic + 1) * 512)
            og_stg, ocp_sb = epi_stg[ic]
            nc.sync.dma_start(ocp_sb[64:128, :, :], og_stg)
            for hp in range(EC):
                nc.vector.tensor_tensor(
                    og_sb[:, hp, isl], ocp_sb[:, hp, :], gp_sb[:, hp, isl], MUL
                )
            out_sbs[ic] = odp.tile(
                [P, 4, CQ], F32, name="out_sb", tag="outsb", bufs=2
            )

        def emit_epilogue_ip(ic, ip4, tail=False):
            """one 128-query chunk of the output projection: K=256 in 64-row
            quarters; T0 chains the lo halves, T8 the hi, banks summed."""
            out_sb = out_sbs[ic]
            ip = ic * 4 + ip4
            slab = psum.tile([P, 2, 512], F32, tag="s", name="ps_o")
            for ec in range(EC):
                nc.tensor.matmul(
                    slab[:, 0, :],
                    _r(og_sb[0:64, ec, ip * P : (ip + 1) * P]),
                    _r(woT_sb[0:64, ec, :]),
                    start=(ec == 0), stop=(ec == EC - 1),
                )
                nc.tensor.matmul(
                    slab[:, 1, :],
                    _r(og_sb[64:128, ec, ip * P : (ip + 1) * P]),
                    _r(woT_sb[64:128, ec, :]),
                    start=(ec == 0), stop=(ec == EC - 1),
                )
            ohi2 = workp.tile([P, 512], F32, name="ohi2", tag="ohi2")
            if tail:
                nc.scalar.copy(ohi2, slab[:, 1, :])
            else:
                nc.vector.tensor_copy(out=ohi2, in_=slab[:, 1, :])
            nc.vector.tensor_tensor(
                out_sb[:, ip4, :], slab[:, 0, :], ohi2, ADD
            )
            if ip4 == 3:
                nc.sync.dma_start(outr[ic], out_sb)

        emit_eb_fetch(0)
        emit_attention(0, 0)
        emit_eb_fetch(1)
        emit_attention(0, 1)
        emit_attention(1, 0)
        pieces = [
            lambda: emit_epilogue_head(0, 0),
            lambda: emit_epilogue_head(0, 1),
            lambda: emit_epilogue_head(0, 2),
            lambda: emit_epilogue_head(0, 3),
            lambda: emit_epilogue_og(0),
            lambda: emit_epilogue_ip(0, 0),
            lambda: emit_epilogue_ip(0, 1),
            lambda: emit_epilogue_ip(0, 2),
            lambda: emit_epilogue_ip(0, 3),
            lambda: emit_epilogue_head(1, 0),
            lambda: emit_epilogue_head(1, 1),
        ]
        emit_attention(1, 1, tail=True, interleave=pieces)
        emit_epilogue_head(1, 2)
        emit_epilogue_head(1, 3)
        emit_epilogue_og(1)
        for ip4 in range(4):
            emit_epilogue_ip(1, ip4, tail=True)


_CACHE = {}


def _get_nc():
    if "nc" not in _CACHE:
        nc = bacc.Bacc("TRN2", debug=False, enable_asserts=False)
        xt = nc.dram_tensor(
            "xt_in", [Q // 512, CC, P, 512], F32R, kind="ExternalInput"
        ).ap()
        ebt = nc.dram_tensor(
            "eb_in", [NI, NG, P, 4 * 512], BF16, kind="ExternalInput"
        ).ap()
        wt = nc.dram_tensor(
            "wt_in", [4, CC, P, EL], F32R, kind="ExternalInput"
        ).ap()
        wot = nc.dram_tensor(
            "wot_in", [P, EC, CQ], F32R, kind="ExternalInput"
        ).ap()
        bg = nc.dram_tensor("bg_in", [P, EC], F32, kind="ExternalInput").ap()
        sel_in = nc.dram_tensor("sel_in", [P, P], F32R, kind="ExternalInput").ap()
        outp = nc.dram_tensor(
            "out", [NI, P, 4, CQ], F32, kind="ExternalOutput"
        ).ap()
        with tile.TileContext(nc) as tc:
            _emit(tc, xt, ebt, wt, wot, bg, sel_in, outp)
        nc.compile()
        _CACHE["nc"] = nc
    return _CACHE["nc"]


LAST_RESULTS = None


def kernel(q_x, kv_x, bias, w_qkv, w_o, b_o, w_g, b_g):
    global LAST_RESULTS
    q_x = np.asarray(q_x, np.float32)
    bias = np.asarray(bias, np.float32)
    w_qkv = np.asarray(w_qkv, np.float32)
    w_o = np.asarray(w_o, np.float32)
    b_o = np.asarray(b_o, np.float32)
    w_g = np.asarray(w_g, np.float32)
    b_g = np.asarray(b_g, np.float32)

    # selection matrix: row 64 ones (reciprocal broadcast); also the warmup
    # stationary (values irrelevant there)
    sel = np.zeros((P, P), np.float32)
    sel[64, :] = 1.0
    in_maps = []
    for core in range(8):
        b, qh, hq = core >> 2, (core >> 1) & 1, core & 1
        i0 = qh * QL
        esl = slice(hq * EL, (hq + 1) * EL)
        xTb = q_x[b].T  # [512, 2048]
        # roll keys so this core's queries are columns 0:QL
        xTp = np.concatenate([xTb[:, i0:], xTb[:, :i0]], axis=1)
        # pre-tile: [j4, c, p, 512] with rows = chans c*128+p
        xtp = np.ascontiguousarray(
            xTp.reshape(CC, P, 4, 512).transpose(2, 0, 1, 3), np.float32
        )
        biasTb = bias[b, 0].T  # [keys, queries]
        ebp = np.exp(
            np.concatenate(
                [biasTb[i0:, i0 : i0 + QL], biasTb[:i0, i0 : i0 + QL]], axis=0
            )
        ).astype(ml_dtypes.bfloat16)
        # pre-tile: [ic, g, p, jj*512+qq], keys = (4g+jj)*128 + p
        ebtp = np.ascontiguousarray(
            ebp.reshape(NG, 4, P, NI, 512)
            .transpose(3, 0, 2, 1, 4)
            .reshape(NI, NG, P, 4 * 512)
        )
        wq = w_qkv[0:CQ][esl] * (1.0 / np.sqrt(D))
        wk = w_qkv[CQ : 2 * CQ][esl]
        wv = w_qkv[2 * CQ : 3 * CQ][esl]
        wg = w_g[esl]
        # pre-tile: [quarter(K,Q,V,G), c, p, e] with rows = chans c*128+p
        wtp = np.ascontiguousarray(
            np.stack(
                [w.T.reshape(CC, P, EL) for w in (wk, wq, wv, wg)], axis=0
            ),
            np.float32,
        )
        # woT pre-tiled: [p, o, c] with e-dim = o*128+p
        woTc = w_o[:, esl].T  # [256, 512]
        wotp = np.ascontiguousarray(
            woTc.reshape(EC, P, CQ).transpose(1, 0, 2), np.float32
        )
        bgc = np.ascontiguousarray(
            (0.5 * b_g[esl]).reshape(EC, P).T, np.float32
        )
        in_maps.append(
            {
                "xt_in": xtp,
                "eb_in": ebtp,
                "wt_in": wtp,
                "wot_in": wotp,
                "bg_in": bgc,
                "sel_in": sel,
            }
        )

    nc = _get_nc()
    res = run_bass_kernel_spmd(nc, in_maps, core_ids=list(range(8)))
    LAST_RESULTS = res

    out = np.zeros((B, Q, CQ), np.float32)
    for core in range(8):
        b, qh = core >> 2, (core >> 1) & 1
        i0 = qh * QL
        # out tensor is [NI, P, 4, CQ]: q row = ic*512 + o*128 + p
        arr = res.results[core]["out"]
        out[b, i0 : i0 + QL] += arr.transpose(0, 2, 1, 3).reshape(QL, CQ)
    out += b_o
    return out
